# revision 26
# baseline (speedup 1.0000x reference)
"""Trainium2 Bass kernel for single-head base attention.

Problem: x [4, 2048, 1024] fp32; Wq/Wk/Wv [1024, 1024] (torch [out, in]).
  Q = x @ Wq.T ; K = x @ Wk.T ; V = x @ Wv.T
  out = softmax(Q K^T / 32) V

Sharding: 8 cores = 4 batches x 2 query-halves. Each core computes K/V for
its batch's full 2048-seq and Q for its 1024-query half; outputs are
disjoint [1024, 1024] slices, so no collectives.

Per-core schedule (all matmuls float32r = tf32-rate, fp32 accumulate):
  phase K: KT[e,k] = WkT.T @ xT      -> resident SBUF (8MB)
  phase V: V[k,e]  = xT.T @ WvT      -> resident SBUF (8MB)
  phase Q: QT[e,q] = WqT.T @ xqT     -> resident SBUF (4MB)
           (wq streamed per e-column, xq streamed per 512-query block,
            so peak SBUF stays under the 192KB/partition cap)
  attention per 128-query tile:
      S[q,k] = QT_sl.T @ KT    in 4 PSUM chunks of [128,512]
      attU chunk = exp(S_chunk/32) on ACT (accum_out = partial row sums;
            no max-subtraction needed: |S/32| <= ~6, exp fits fp32 easily)
      attT = PE-transpose(attU) 128x128 blocks, chasing the exp chunks
      O[q,e] = attT.T @ V      (PSUM accumulate over k)
      out = O * (1/sum exp)    -> DRAM
"""

import os
import sys
from contextlib import ExitStack

import numpy as np

for _p in ("/opt/trn_rl_repo", "/root/.axon_site/_ro/trn_rl_repo"):
    if os.path.isdir(_p) and _p not in sys.path:
        sys.path.append(_p)

import concourse.bass as bass
import concourse.mybir as mybir
from concourse import bacc, tile
from concourse.bass_utils import run_bass_kernel_spmd

F32 = mybir.dt.float32
F32R = mybir.dt.float32r  # tf32-rate matmul dtype, fp32 storage bits

B, SEQ, D = 4, 2048, 1024
QL = SEQ // 2          # queries per core
N_CORES = 8
DT = D // 128          # 8 d-tiles (contraction)
ET = D // 128          # 8 e-tiles (hidden out)
KT = SEQ // 128        # 16 k-tiles
QT = QL // 128         # 8 q-tiles per core
XB = 256               # xT streaming col-block width
NXB = SEQ // XB        # 8 blocks
AF = mybir.ActivationFunctionType


def _copy(nc, i, dst, src):
    # alternate PSUM->SBUF copies between DVE and ACT to balance engines
    if i % 2 == 0:
        nc.vector.tensor_copy(dst, src)
    else:
        nc.scalar.copy(dst, src)


def build(reps: int = 1):
    nc = bacc.Bacc(
        "TRN2", target_bir_lowering=False, debug=False, num_devices=N_CORES
    )

    xqT = nc.declare_dram_parameter("xqT", [D, QL], F32R, isOutput=False)
    WqT = nc.declare_dram_parameter("WqT", [D, D], F32R, isOutput=False)
    WkT = nc.declare_dram_parameter("WkT", [D, D], F32R, isOutput=False)
    WvT = nc.declare_dram_parameter("WvT", [D, D], F32R, isOutput=False)
    idn = nc.declare_dram_parameter("idn", [128, 128], F32R, isOutput=False)
    out = nc.declare_dram_parameter("out", [QL, D], F32, isOutput=True)

    aps = {
        "xqT_r": xqT.rearrange("(dt p) q -> p dt q", p=128),
        "Wq_r": WqT.rearrange("(dt p) e -> p dt e", p=128),
        "Wk_r": WkT.rearrange("(dt p) e -> p dt e", p=128),
        "Wv_r": WvT.rearrange("(dt p) e -> p dt e", p=128),
        "out_r": out.rearrange("(qt p) e -> qt p e", p=128),
    }

    with ExitStack() as top:
        tc = top.enter_context(tile.TileContext(nc))

        for _ in range(reps):
            _body(nc, tc, idn, aps)

    nc.compile()
    return nc


def _body(nc, tc, idn, aps):
    xqT_r = aps["xqT_r"]
    Wq_r = aps["Wq_r"]
    Wk_r = aps["Wk_r"]
    Wv_r = aps["Wv_r"]
    out_r = aps["out_r"]
    HL = QL  # own half length (1024)

    with ExitStack() as st:
        dram = st.enter_context(tc.tile_pool(name="ccd", bufs=1, space="DRAM"))
        ccK_src = dram.tile([D, HL], F32R, tag="ccK_src", addr_space="Local")
        ccK_dst = dram.tile([2, D, HL], F32R, tag="ccK_dst", addr_space="Local")
        ccV_src = dram.tile([HL, D], F32R, tag="ccV_src", addr_space="Local")
        ccV_dst = dram.tile([2, HL, D], F32R, tag="ccV_dst", addr_space="Local")

        # ---- phase KVh: own-half K^T and V from xqT only ----
        with (
            tc.tile_pool(name="kvh_x", bufs=1) as kvh_x,
            tc.tile_pool(name="kvh_wk", bufs=1) as kvh_wk,
            tc.tile_pool(name="kvh_wv", bufs=1) as kvh_wv,
            tc.tile_pool(name="kvh_s", bufs=4) as kvh_s,
            tc.tile_pool(name="psh", bufs=4, space="PSUM") as psh,
        ):
            xq = kvh_x.tile([128, DT, HL], F32R, tag="xq")
            wv = kvh_wv.tile([128, DT, D], F32R, tag="wv")
            wk = kvh_wk.tile([128, DT, D], F32R, tag="wk")
            # issue order matters for the head: Vh's first group needs only
            # the first xq half + first wv chunk
            nc.sync.dma_start(xq[:, :, :512], xqT_r[:, :, :512])
            nc.sync.dma_start(wv[:], Wv_r[:])
            nc.sync.dma_start(xq[:, :, 512:], xqT_r[:, :, 512:])
            nc.sync.dma_start(wk[:], Wk_r[:])

            ci = 0
            # Vh[k_own, e] = xq.T @ WvT
            for kt2 in range(HL // 128):
                for ec in range(D // 512):
                    ps = psh.tile([128, 512], F32, tag="ps")
                    for d in range(DT):
                        nc.tensor.matmul(
                            ps[:],
                            xq[:, d, kt2 * 128 : (kt2 + 1) * 128],
                            wv[:, d, ec * 512 : (ec + 1) * 512],
                            start=(d == 0),
                            stop=(d == DT - 1),
                        )
                    piece = kvh_s.tile([128, 512], F32R, tag="piece")
                    _copy(nc, ci, piece[:], ps[:])
                    ci += 1
                    nc.sync.dma_start(
                        ccV_src[kt2 * 128 : (kt2 + 1) * 128, ec * 512 : (ec + 1) * 512],
                        piece[:],
                    )
            nc.gpsimd.collective_compute(
                "AllGather",
                mybir.AluOpType.bypass,
                replica_groups=[[0, 1], [2, 3], [4, 5], [6, 7]],
                ins=[ccV_src[:]],
                outs=[ccV_dst[:]],
            )

            # KTh[e, k_own] = WkT.T @ xq
            for kc in range(HL // 512):
                for et in range(ET):
                    ps = psh.tile([128, 512], F32, tag="ps")
                    for d in range(DT):
                        nc.tensor.matmul(
                            ps[:],
                            wk[:, d, et * 128 : (et + 1) * 128],
                            xq[:, d, kc * 512 : (kc + 1) * 512],
                            start=(d == 0),
                            stop=(d == DT - 1),
                        )
                    piece = kvh_s.tile([128, 512], F32R, tag="piece")
                    _copy(nc, ci, piece[:], ps[:])
                    ci += 1
                    nc.sync.dma_start(
                        ccK_src[et * 128 : (et + 1) * 128, kc * 512 : (kc + 1) * 512],
                        piece[:],
                    )
            nc.gpsimd.collective_compute(
                "AllGather",
                mybir.AluOpType.bypass,
                replica_groups=[[0, 1], [2, 3], [4, 5], [6, 7]],
                ins=[ccK_src[:]],
                outs=[ccK_dst[:]],
            )

        # ---- load gathered K/V into residents ----
        res_kv = st.enter_context(tc.tile_pool(name="res_kv", bufs=1))
        kt_sb = res_kv.tile([128, ET, SEQ], F32R, tag="kt_sb")
        v_sb = res_kv.tile([128, KT, D], F32R, tag="v_sb")
        # rank0 = even core = first query half, so concatenating rank halves
        # reproduces the original key order exactly.
        for r in range(2):
            nc.sync.dma_start(
                v_sb[:, r * 8 : (r + 1) * 8, :],
                ccV_dst[r].rearrange("(kt p) e -> p kt e", p=128),
            )
            nc.sync.dma_start(
                kt_sb[:, :, r * HL : (r + 1) * HL],
                ccK_dst[r].rearrange("(et p) k -> p et k", p=128),
            )

        # ---------------- phase Q: QT[e,q] resident ----------------
        with tc.tile_pool(name="res2", bufs=1) as res2:
            qt_sb = res2.tile([128, ET, QL], F32R, tag="qt_sb")
            with (
                tc.tile_pool(name="phq_w", bufs=3) as phq_w,
                tc.tile_pool(name="phq_x", bufs=2) as phq_x,
                tc.tile_pool(name="psq", bufs=4, space="PSUM") as psq,
            ):
                ci = 0
                for qc in range(QL // 512):
                    xqb = phq_x.tile([128, DT, 512], F32R, tag="xqb")
                    nc.sync.dma_start(
                        xqb[:], xqT_r[:, :, qc * 512 : (qc + 1) * 512]
                    )
                    for et in range(ET):
                        wqc = phq_w.tile([128, DT, 128], F32R, tag="wqc")
                        nc.sync.dma_start(
                            wqc[:], Wq_r[:, :, et * 128 : (et + 1) * 128]
                        )
                        ps = psq.tile([128, 512], F32, tag="ps")
                        for d in range(DT):
                            nc.tensor.matmul(
                                ps[:],
                                wqc[:, d, :],
                                xqb[:, d, :],
                                start=(d == 0),
                                stop=(d == DT - 1),
                            )
                        _copy(nc, ci, qt_sb[:, et, qc * 512 : (qc + 1) * 512], ps[:])
                        ci += 1

            # ---------------- attention ----------------
            with (
                tc.tile_pool(name="attu_p", bufs=2) as attu_p,
                tc.tile_pool(name="attt_p", bufs=4) as attt_p,
                tc.tile_pool(name="osb_p", bufs=2) as osb_p,
                tc.tile_pool(name="vec_p", bufs=16) as vec_p,
                tc.tile_pool(name="pss", bufs=4, space="PSUM") as pss,
                tc.tile_pool(name="pst", bufs=2, space="PSUM") as pst,
                tc.tile_pool(name="pso", bufs=2, space="PSUM") as pso,
                tc.tile_pool(name="const", bufs=1) as const_pool,
            ):
                ident = const_pool.tile([128, 128], F32R)
                nc.sync.dma_start(ident[:], idn[:])
                NKC = SEQ // 512  # 4 S-chunks per q-tile
                for qt in range(QT):
                    schunks = []
                    for kc in range(NKC):
                        Skc = pss.tile([128, 512], F32, tag="S")
                        for et in range(ET):
                            nc.tensor.matmul(
                                Skc[:],
                                qt_sb[:, et, qt * 128 : (qt + 1) * 128],
                                kt_sb[:, et, kc * 512 : (kc + 1) * 512],
                                start=(et == 0),
                                stop=(et == ET - 1),
                            )
                        schunks.append(Skc)
                    attu = attu_p.tile([128, SEQ], F32R, tag="attu")
                    rsp = []
                    for kc in range(NKC):
                        rs = vec_p.tile([128, 1], F32, tag="rs")
                        nc.scalar.activation(
                            attu[:, kc * 512 : (kc + 1) * 512],
                            schunks[kc][:],
                            AF.Exp,
                            scale=1.0 / 32.0,
                            accum_out=rs[:],
                        )
                        rsp.append(rs)
                    rs01 = vec_p.tile([128, 1], F32, tag="rx")
                    rs23 = vec_p.tile([128, 1], F32, tag="rx")
                    rsum = vec_p.tile([128, 1], F32, tag="rx")
                    nc.vector.tensor_add(rs01[:], rsp[0][:], rsp[1][:])
                    nc.vector.tensor_add(rs23[:], rsp[2][:], rsp[3][:])
                    nc.vector.tensor_add(rsum[:], rs01[:], rs23[:])
                    r = vec_p.tile([128, 1], F32, tag="rx")
                    nc.vector.reciprocal(r[:], rsum[:])

                    po0 = pso.tile([128, 512], F32, tag="po")
                    po1 = pso.tile([128, 512], F32, tag="po")
                    pos = (po0, po1)
                    for kt in range(KT):
                        tp = pst.tile([128, 128], F32R, tag="tp")
                        nc.tensor.transpose(
                            tp[:], attu[:, kt * 128 : (kt + 1) * 128], ident[:]
                        )
                        at = attt_p.tile([128, 128], F32R, tag="at")
                        nc.vector.tensor_copy(at[:], tp[:])
                        for ec in range(2):
                            nc.tensor.matmul(
                                pos[ec][:],
                                at[:],
                                v_sb[:, kt, ec * 512 : (ec + 1) * 512],
                                start=(kt == 0),
                                stop=(kt == KT - 1),
                            )
                    osb = osb_p.tile([128, D], F32, tag="osb")
                    for ec in range(2):
                        nc.vector.tensor_scalar_mul(
                            osb[:, ec * 512 : (ec + 1) * 512], pos[ec][:], r[:]
                        )
                    nc.sync.dma_start(out_r[qt], osb[:])


_CACHE: dict = {}


def _get_nc():
    if "nc" not in _CACHE:
        _CACHE["nc"] = build()
    return _CACHE["nc"]


def _get_runner():
    """Cached jitted shard_map executable over the 8 cores.

    Mirrors concourse.bass2jax.run_bass_via_pjrt but builds the jit once,
    so repeated kernel() calls only pay input transfer + execute.
    """
    if "runner" in _CACHE:
        return _CACHE["runner"]

    import jax
    from jax.sharding import Mesh, NamedSharding, PartitionSpec
    from jax.experimental.shard_map import shard_map

    from concourse import bass2jax, mybir as _mybir

    nc = _get_nc()
    bass2jax.install_neuronx_cc_hook()

    partition_name = (
        nc.partition_id_tensor.name if nc.partition_id_tensor else None
    )
    in_names = []
    out_names = []
    out_avals = []
    zero_outs = []
    for alloc in nc.m.functions[0].allocations:
        if not isinstance(alloc, _mybir.MemoryLocationSet):
            continue
        if alloc.kind == "ExternalInput":
            if alloc.memorylocations[0].name == partition_name:
                continue
            in_names.append(alloc.memorylocations[0].name)
        elif alloc.kind == "ExternalOutput":
            name = alloc.memorylocations[0].name
            out_names.append(name)
            shape = tuple(alloc.tensor_shape)
            dtype = _mybir.dt.np(alloc.dtype)
            out_avals.append(jax.core.ShapedArray(shape, dtype))
            zero_outs.append(np.zeros(shape, dtype))
    n_params = len(in_names)
    all_in_names = in_names + out_names
    if partition_name is not None:
        all_in_names = all_in_names + [partition_name]

    def _body_fn(*args):
        operands = list(args)
        if partition_name is not None:
            operands.append(bass2jax.partition_id_tensor())
        outs = bass2jax._bass_exec_p.bind(
            *operands,
            out_avals=tuple(out_avals),
            in_names=tuple(all_in_names),
            out_names=tuple(out_names),
            lowering_input_output_aliases=(),
            sim_require_finite=True,
            sim_require_nnan=True,
            nc=nc,
        )
        return tuple(outs)

    devices = jax.devices()[:N_CORES]
    mesh = Mesh(np.asarray(devices), ("core",))
    spec = PartitionSpec("core")
    n_outs = len(out_names)
    # No donation: the kernel writes every element of "out", so results
    # don't need to alias the zero placeholders. This lets callers reuse
    # the same device-resident placeholder arrays across calls.
    sharded = jax.jit(
        shard_map(
            _body_fn,
            mesh=mesh,
            in_specs=(spec,) * (n_params + n_outs),
            out_specs=(spec,) * n_outs,
            check_rep=False,
        ),
        keep_unused=True,
    )
    sharding = NamedSharding(mesh, spec)

    def run(in_maps):
        concat_in = [
            np.concatenate([np.asarray(m[name]) for m in in_maps], axis=0)
            for name in in_names
        ]
        concat_zeros = [
            np.zeros((N_CORES * z.shape[0], *z.shape[1:]), z.dtype)
            for z in zero_outs
        ]
        dev_in = [jax.device_put(a, sharding) for a in concat_in]
        dev_zero = [jax.device_put(a, sharding) for a in concat_zeros]
        out_arrs = sharded(*dev_in, *dev_zero)
        return [
            {
                name: np.asarray(out_arrs[i]).reshape(
                    N_CORES, *out_avals[i].shape
                )[c]
                for i, name in enumerate(out_names)
            }
            for c in range(N_CORES)
        ]

    def run_device(dev_in, dev_zero):
        return sharded(*dev_in, *dev_zero)

    _CACHE["runner"] = (run, run_device, sharding, in_names, zero_outs)
    return _CACHE["runner"]


def _make_in_maps(x, Wq, Wk, Wv):
    x = np.asarray(x, dtype=np.float32)
    wqT = np.ascontiguousarray(np.asarray(Wq, dtype=np.float32).T)
    wkT = np.ascontiguousarray(np.asarray(Wk, dtype=np.float32).T)
    wvT = np.ascontiguousarray(np.asarray(Wv, dtype=np.float32).T)
    eye = np.eye(128, dtype=np.float32)
    in_maps = []
    for c in range(N_CORES):
        b, h = divmod(c, 2)
        xq = np.ascontiguousarray(x[b, h * QL : (h + 1) * QL].T)
        in_maps.append(
            {
                "xqT": xq,
                "WqT": wqT,
                "WkT": wkT,
                "WvT": wvT,
                "idn": eye,
            }
        )
    return in_maps


def _assemble(results):
    out = np.empty((B, SEQ, D), dtype=np.float32)
    for c in range(N_CORES):
        b, h = divmod(c, 2)
        out[b, h * QL : (h + 1) * QL] = results[c]["out"]
    return out


def run_traced(x, Wq, Wk, Wv, **kw):
    """Run via run_bass_kernel_spmd, return (output, BassKernelResults)."""
    nc = _get_nc()
    res = run_bass_kernel_spmd(
        nc, _make_in_maps(x, Wq, Wk, Wv), list(range(N_CORES)), **kw
    )
    return _assemble(res.results), res


def kernel(x, Wq, Wk, Wv):
    run, _, _, _, _ = _get_runner()
    results = run(_make_in_maps(x, Wq, Wk, Wv))
    return _assemble(results)


# revision 28
# speedup vs baseline: 1.2697x; 1.2697x over previous
"""Trainium2 Bass kernel for single-head base attention.

Problem: x [4, 2048, 1024] fp32; Wq/Wk/Wv [1024, 1024] (torch [out, in]).
  Q = x @ Wq.T ; K = x @ Wk.T ; V = x @ Wv.T
  out = softmax(Q K^T / 32) V

Sharding: 8 cores = 4 batches x 2 query-halves. Each core computes K/V for
its batch's full 2048-seq and Q for its 1024-query half; outputs are
disjoint [1024, 1024] slices, so no collectives.

Per-core schedule (all matmuls float32r = tf32-rate, fp32 accumulate):
  phase K: KT[e,k] = WkT.T @ xT      -> resident SBUF (8MB)
  phase V: V[k,e]  = xT.T @ WvT      -> resident SBUF (8MB)
  phase Q: QT[e,q] = WqT.T @ xqT     -> resident SBUF (4MB)
           (wq streamed per e-column, xq streamed per 512-query block,
            so peak SBUF stays under the 192KB/partition cap)
  attention per 128-query tile:
      S[q,k] = QT_sl.T @ KT    in 4 PSUM chunks of [128,512]
      attU chunk = exp(S_chunk/32) on ACT (accum_out = partial row sums;
            no max-subtraction needed: |S/32| <= ~6, exp fits fp32 easily)
      attT = PE-transpose(attU) 128x128 blocks, chasing the exp chunks
      O[q,e] = attT.T @ V      (PSUM accumulate over k)
      out = O * (1/sum exp)    -> DRAM
"""

import os
import sys
from contextlib import ExitStack

import numpy as np

for _p in ("/opt/trn_rl_repo", "/root/.axon_site/_ro/trn_rl_repo"):
    if os.path.isdir(_p) and _p not in sys.path:
        sys.path.append(_p)

import concourse.bass as bass
import concourse.mybir as mybir
from concourse import bacc, tile
from concourse.bass_utils import run_bass_kernel_spmd

F32 = mybir.dt.float32
F32R = mybir.dt.float32r  # tf32-rate matmul dtype, fp32 storage bits

B, SEQ, D = 4, 2048, 1024
QL = SEQ // 2          # queries per core
N_CORES = 8
DT = D // 128          # 8 d-tiles (contraction)
ET = D // 128          # 8 e-tiles (hidden out)
KT = SEQ // 128        # 16 k-tiles
QT = QL // 128         # 8 q-tiles per core
XB = 256               # xT streaming col-block width
NXB = SEQ // XB        # 8 blocks
AF = mybir.ActivationFunctionType


def _copy(nc, i, dst, src):
    # alternate PSUM->SBUF copies between DVE and ACT to balance engines
    if i % 2 == 0:
        nc.vector.tensor_copy(dst, src)
    else:
        nc.scalar.copy(dst, src)


def build(reps: int = 1):
    nc = bacc.Bacc(
        "TRN2", target_bir_lowering=False, debug=False, num_devices=N_CORES
    )

    xT = nc.declare_dram_parameter("xT", [D, SEQ], F32R, isOutput=False)
    xqT = nc.declare_dram_parameter("xqT", [D, QL], F32R, isOutput=False)
    WqT = nc.declare_dram_parameter("WqT", [D, D], F32R, isOutput=False)
    WkT = nc.declare_dram_parameter("WkT", [D, D], F32R, isOutput=False)
    WvT = nc.declare_dram_parameter("WvT", [D, D], F32R, isOutput=False)
    idn = nc.declare_dram_parameter("idn", [128, 128], F32R, isOutput=False)
    out = nc.declare_dram_parameter("out", [QL, D], F32, isOutput=True)

    aps = {
        "xT_r": xT.rearrange("(dt p) k -> p dt k", p=128),
        "xqT_r": xqT.rearrange("(dt p) q -> p dt q", p=128),
        "Wq_r": WqT.rearrange("(dt p) e -> p dt e", p=128),
        "Wk_r": WkT.rearrange("(dt p) e -> p dt e", p=128),
        "Wv_r": WvT.rearrange("(dt p) e -> p dt e", p=128),
        "out_r": out.rearrange("(qt p) e -> qt p e", p=128),
    }

    with ExitStack() as top:
        tc = top.enter_context(tile.TileContext(nc))

        res_pool = top.enter_context(tc.tile_pool(name="res", bufs=1))

        kt_sb = res_pool.tile([128, ET, SEQ], F32R, tag="kt_sb")
        v_sb = res_pool.tile([128, KT, D], F32R, tag="v_sb")

        for _ in range(reps):
            _body(nc, tc, idn, kt_sb, v_sb, aps)

    nc.compile()
    return nc


def _body(nc, tc, idn, kt_sb, v_sb, aps):
    xT_r = aps["xT_r"]
    xqT_r = aps["xqT_r"]
    Wq_r = aps["Wq_r"]
    Wk_r = aps["Wk_r"]
    Wv_r = aps["Wv_r"]
    out_r = aps["out_r"]

    # phv_w outlives phase K so wv can prefetch during K's matmuls
    # (SBUF cap is ~208KB/partition; phase K peaks at ~208.2).
    with tc.tile_pool(name="phv_w", bufs=1) as phv_w:
        # wv split 6/2: the 24KB head prefetches during phase K (fits the
        # 208KB/partition cap), the 8KB tail loads at the start of phase V
        wv_a = phv_w.tile([128, 7, D], F32R, tag="wv_a")

        # ---------------- phase K: KT[e,k] resident ----------------
        ph_x = tc.tile_pool(name="ph_x", bufs=2)
        ph_x_pool = ph_x.__enter__()
        last_xtb = None
        with (
            tc.tile_pool(name="phk_w", bufs=1) as phk_w,
            tc.tile_pool(name="psk", bufs=4, space="PSUM") as psk,
        ):
            # wk in two halves so the first matmuls start after 3MB of DMA
            # (xtb block 0 + wk_a) instead of 5MB
            wk_a = phk_w.tile([128, 4, D], F32R, tag="wk_a")
            wk_b = phk_w.tile([128, 4, D], F32R, tag="wk_b")

            def wkd(d, et):
                half = wk_a if d < 4 else wk_b
                return half[:, d % 4, et * 128 : (et + 1) * 128]

            ci = 0
            for j in range(NXB):
                xtb = ph_x_pool.tile([128, DT, XB], F32R, tag="xtb")
                nc.sync.dma_start(xtb[:], xT_r[:, :, j * XB : (j + 1) * XB])
                if j == 0:
                    nc.sync.dma_start(wk_a[:], Wk_r[:, :4, :])
                    nc.sync.dma_start(wk_b[:], Wk_r[:, 4:, :])
                if j == 2:
                    # ~40us of K matmuls remain: hides the 3MB wv_a load
                    nc.sync.dma_start(wv_a[:], Wv_r[:, :7, :])
                if j == 0:
                    # split-accumulation warm-up: run d 0-3 of the first four
                    # groups on wk_a alone, then finish with wk_b
                    pss0 = []
                    for et in range(4):
                        ps = psk.tile([128, XB], F32, tag="ps")
                        for d in range(4):
                            nc.tensor.matmul(
                                ps[:], wkd(d, et), xtb[:, d, :],
                                start=(d == 0), stop=False,
                            )
                        pss0.append(ps)
                    for et in range(4):
                        ps = pss0[et]
                        for d in range(4, DT):
                            nc.tensor.matmul(
                                ps[:], wkd(d, et), xtb[:, d, :],
                                start=False, stop=(d == DT - 1),
                            )
                        _copy(nc, ci, kt_sb[:, et, j * XB : (j + 1) * XB], ps[:])
                        ci += 1
                    rest = range(4, ET)
                else:
                    rest = range(ET)
                for et in rest:
                    ps = psk.tile([128, XB], F32, tag="ps")
                    for d in range(DT):
                        nc.tensor.matmul(
                            ps[:], wkd(d, et), xtb[:, d, :],
                            start=(d == 0), stop=(d == DT - 1),
                        )
                    _copy(nc, ci, kt_sb[:, et, j * XB : (j + 1) * XB], ps[:])
                    ci += 1

        # ---------------- phase V: V[k,e] resident ----------------
        with (
            tc.tile_pool(name="phv_wb", bufs=1) as phv_wb,
            tc.tile_pool(name="psv", bufs=4, space="PSUM") as psv,
        ):
            wv_b = phv_wb.tile([128, 1, D], F32R, tag="wv_b")
            nc.sync.dma_start(wv_b[:], Wv_r[:, 7:, :])
            ci = 0
            # reverse order: block NXB-1 is still resident from phase K
            for j in range(NXB - 1, -1, -1):
                if j == NXB - 1 and last_xtb is not None:
                    xtb = last_xtb
                else:
                    xtb = ph_x_pool.tile([128, DT, XB], F32R, tag="xtb")
                    nc.sync.dma_start(xtb[:], xT_r[:, :, j * XB : (j + 1) * XB])
                for k2 in range(XB // 128):
                    kt = j * (XB // 128) + k2
                    for ec in range(D // 512):
                        ps = psv.tile([128, 512], F32, tag="ps")
                        for d in range(DT):
                            wvd = (
                                wv_a[:, d, ec * 512 : (ec + 1) * 512]
                                if d < 7
                                else wv_b[:, d - 7, ec * 512 : (ec + 1) * 512]
                            )
                            nc.tensor.matmul(
                                ps[:],
                                xtb[:, d, k2 * 128 : (k2 + 1) * 128],
                                wvd,
                                start=(d == 0),
                                stop=(d == DT - 1),
                            )
                        _copy(nc, ci, v_sb[:, kt, ec * 512 : (ec + 1) * 512], ps[:])
                        ci += 1
        ph_x.__exit__(None, None, None)

    # ---------------- phase Q: QT[e,q] resident ----------------
    # wq streamed per e-column (0.5MB x2 bufs) and xq per 512-query block.
    with tc.tile_pool(name="res2", bufs=1) as res2:
        qt_sb = res2.tile([128, ET, QL], F32R, tag="qt_sb")
        with (
            tc.tile_pool(name="phq_w", bufs=3) as phq_w,
            tc.tile_pool(name="phq_x", bufs=2) as phq_x,
            tc.tile_pool(name="psq", bufs=4, space="PSUM") as psq,
        ):
            ci = 0
            for qc in range(QL // 512):
                xq = phq_x.tile([128, DT, 512], F32R, tag="xq")
                nc.sync.dma_start(xq[:], xqT_r[:, :, qc * 512 : (qc + 1) * 512])
                for et in range(ET):
                    wqc = phq_w.tile([128, DT, 128], F32R, tag="wqc")
                    nc.sync.dma_start(
                        wqc[:], Wq_r[:, :, et * 128 : (et + 1) * 128]
                    )
                    ps = psq.tile([128, 512], F32, tag="ps")
                    for d in range(DT):
                        nc.tensor.matmul(
                            ps[:],
                            wqc[:, d, :],
                            xq[:, d, :],
                            start=(d == 0),
                            stop=(d == DT - 1),
                        )
                    _copy(nc, ci, qt_sb[:, et, qc * 512 : (qc + 1) * 512], ps[:])
                    ci += 1

        # ---------------- attention ----------------
        with (
            tc.tile_pool(name="attu_p", bufs=2) as attu_p,
            tc.tile_pool(name="attt_p", bufs=4) as attt_p,
            tc.tile_pool(name="osb_p", bufs=2) as osb_p,
            tc.tile_pool(name="vec_p", bufs=16) as vec_p,
            tc.tile_pool(name="pss", bufs=4, space="PSUM") as pss,
            tc.tile_pool(name="pst", bufs=2, space="PSUM") as pst,
            tc.tile_pool(name="pso", bufs=2, space="PSUM") as pso,
            tc.tile_pool(name="const", bufs=1) as const_pool,
        ):
            ident = const_pool.tile([128, 128], F32R)
            nc.sync.dma_start(ident[:], idn[:])
            NKC = SEQ // 512  # 4 S-chunks per q-tile
            for qt in range(QT):
                schunks = []
                for kc in range(NKC):
                    Skc = pss.tile([128, 512], F32, tag="S")
                    for et in range(ET):
                        nc.tensor.matmul(
                            Skc[:],
                            qt_sb[:, et, qt * 128 : (qt + 1) * 128],
                            kt_sb[:, et, kc * 512 : (kc + 1) * 512],
                            start=(et == 0),
                            stop=(et == ET - 1),
                        )
                    schunks.append(Skc)
                attu = attu_p.tile([128, SEQ], F32R, tag="attu")
                rsp = []
                for kc in range(NKC):
                    rs = vec_p.tile([128, 1], F32, tag="rs")
                    nc.scalar.activation(
                        attu[:, kc * 512 : (kc + 1) * 512],
                        schunks[kc][:],
                        AF.Exp,
                        scale=1.0 / 32.0,
                        accum_out=rs[:],
                    )
                    rsp.append(rs)
                rs01 = vec_p.tile([128, 1], F32, tag="rx")
                rs23 = vec_p.tile([128, 1], F32, tag="rx")
                rsum = vec_p.tile([128, 1], F32, tag="rx")
                nc.vector.tensor_add(rs01[:], rsp[0][:], rsp[1][:])
                nc.vector.tensor_add(rs23[:], rsp[2][:], rsp[3][:])
                nc.vector.tensor_add(rsum[:], rs01[:], rs23[:])
                r = vec_p.tile([128, 1], F32, tag="rx")
                nc.vector.reciprocal(r[:], rsum[:])

                po0 = pso.tile([128, 512], F32, tag="po")
                po1 = pso.tile([128, 512], F32, tag="po")
                pos = (po0, po1)
                for kt in range(KT):
                    tp = pst.tile([128, 128], F32R, tag="tp")
                    nc.tensor.transpose(
                        tp[:], attu[:, kt * 128 : (kt + 1) * 128], ident[:]
                    )
                    at = attt_p.tile([128, 128], F32R, tag="at")
                    # attention-critical copies stay on DVE so ACT only
                    # runs the exp chain here
                    nc.vector.tensor_copy(at[:], tp[:])
                    for ec in range(2):
                        nc.tensor.matmul(
                            pos[ec][:],
                            at[:],
                            v_sb[:, kt, ec * 512 : (ec + 1) * 512],
                            start=(kt == 0),
                            stop=(kt == KT - 1),
                        )
                osb = osb_p.tile([128, D], F32, tag="osb")
                for ec in range(2):
                    nc.vector.tensor_scalar_mul(
                        osb[:, ec * 512 : (ec + 1) * 512], pos[ec][:], r[:]
                    )
                nc.sync.dma_start(out_r[qt], osb[:])


_CACHE: dict = {}


def _get_nc():
    if "nc" not in _CACHE:
        _CACHE["nc"] = build()
    return _CACHE["nc"]


def _get_runner():
    """Cached jitted shard_map executable over the 8 cores.

    Mirrors concourse.bass2jax.run_bass_via_pjrt but builds the jit once,
    so repeated kernel() calls only pay input transfer + execute.
    """
    if "runner" in _CACHE:
        return _CACHE["runner"]

    import jax
    from jax.sharding import Mesh, NamedSharding, PartitionSpec
    from jax.experimental.shard_map import shard_map

    from concourse import bass2jax, mybir as _mybir

    nc = _get_nc()
    bass2jax.install_neuronx_cc_hook()

    partition_name = (
        nc.partition_id_tensor.name if nc.partition_id_tensor else None
    )
    in_names = []
    out_names = []
    out_avals = []
    zero_outs = []
    for alloc in nc.m.functions[0].allocations:
        if not isinstance(alloc, _mybir.MemoryLocationSet):
            continue
        if alloc.kind == "ExternalInput":
            if alloc.memorylocations[0].name == partition_name:
                continue
            in_names.append(alloc.memorylocations[0].name)
        elif alloc.kind == "ExternalOutput":
            name = alloc.memorylocations[0].name
            out_names.append(name)
            shape = tuple(alloc.tensor_shape)
            dtype = _mybir.dt.np(alloc.dtype)
            out_avals.append(jax.core.ShapedArray(shape, dtype))
            zero_outs.append(np.zeros(shape, dtype))
    n_params = len(in_names)
    all_in_names = in_names + out_names
    if partition_name is not None:
        all_in_names = all_in_names + [partition_name]

    def _body_fn(*args):
        operands = list(args)
        if partition_name is not None:
            operands.append(bass2jax.partition_id_tensor())
        outs = bass2jax._bass_exec_p.bind(
            *operands,
            out_avals=tuple(out_avals),
            in_names=tuple(all_in_names),
            out_names=tuple(out_names),
            lowering_input_output_aliases=(),
            sim_require_finite=True,
            sim_require_nnan=True,
            nc=nc,
        )
        return tuple(outs)

    devices = jax.devices()[:N_CORES]
    mesh = Mesh(np.asarray(devices), ("core",))
    spec = PartitionSpec("core")
    n_outs = len(out_names)
    # No donation: the kernel writes every element of "out", so results
    # don't need to alias the zero placeholders. This lets callers reuse
    # the same device-resident placeholder arrays across calls.
    sharded = jax.jit(
        shard_map(
            _body_fn,
            mesh=mesh,
            in_specs=(spec,) * (n_params + n_outs),
            out_specs=(spec,) * n_outs,
            check_rep=False,
        ),
        keep_unused=True,
    )
    sharding = NamedSharding(mesh, spec)

    def run(in_maps):
        concat_in = [
            np.concatenate([np.asarray(m[name]) for m in in_maps], axis=0)
            for name in in_names
        ]
        concat_zeros = [
            np.zeros((N_CORES * z.shape[0], *z.shape[1:]), z.dtype)
            for z in zero_outs
        ]
        dev_in = [jax.device_put(a, sharding) for a in concat_in]
        dev_zero = [jax.device_put(a, sharding) for a in concat_zeros]
        out_arrs = sharded(*dev_in, *dev_zero)
        return [
            {
                name: np.asarray(out_arrs[i]).reshape(
                    N_CORES, *out_avals[i].shape
                )[c]
                for i, name in enumerate(out_names)
            }
            for c in range(N_CORES)
        ]

    def run_device(dev_in, dev_zero):
        return sharded(*dev_in, *dev_zero)

    _CACHE["runner"] = (run, run_device, sharding, in_names, zero_outs)
    return _CACHE["runner"]


def _make_in_maps(x, Wq, Wk, Wv):
    x = np.asarray(x, dtype=np.float32)
    wqT = np.ascontiguousarray(np.asarray(Wq, dtype=np.float32).T)
    wkT = np.ascontiguousarray(np.asarray(Wk, dtype=np.float32).T)
    wvT = np.ascontiguousarray(np.asarray(Wv, dtype=np.float32).T)
    eye = np.eye(128, dtype=np.float32)
    in_maps = []
    for c in range(N_CORES):
        b, h = divmod(c, 2)
        xb = np.ascontiguousarray(x[b].T)
        xq = np.ascontiguousarray(x[b, h * QL : (h + 1) * QL].T)
        in_maps.append(
            {
                "xT": xb,
                "xqT": xq,
                "WqT": wqT,
                "WkT": wkT,
                "WvT": wvT,
                "idn": eye,
            }
        )
    return in_maps


def _assemble(results):
    out = np.empty((B, SEQ, D), dtype=np.float32)
    for c in range(N_CORES):
        b, h = divmod(c, 2)
        out[b, h * QL : (h + 1) * QL] = results[c]["out"]
    return out


def run_traced(x, Wq, Wk, Wv, **kw):
    """Run via run_bass_kernel_spmd, return (output, BassKernelResults)."""
    nc = _get_nc()
    res = run_bass_kernel_spmd(
        nc, _make_in_maps(x, Wq, Wk, Wv), list(range(N_CORES)), **kw
    )
    return _assemble(res.results), res


def kernel(x, Wq, Wk, Wv):
    run, _, _, _, _ = _get_runner()
    results = run(_make_in_maps(x, Wq, Wk, Wv))
    return _assemble(results)
